# revision 9
# baseline (speedup 1.0000x reference)
"""Trainium2 Bass kernel for the CML sparse-attention fusion block.

Contract: kernel(**inputs) takes the FULL inputs from setup_inputs() and
returns the full (rgb_full, th_full) outputs, distributing work across 8
NeuronCores internally.

Sharding: 8 cores = 4 batch elements x 2 modalities. Core c computes the
fused output for batch b = c // 2, modality m = c % 2 (m=0 -> fuse_rgb,
m=1 -> fuse_th). Each core's work (one SNA block + MLP over 2048 sparse
queries) is fully independent -> pure SPMD, no collectives.

Device pipeline per core (all f32):
  A: full-grid LN stats of x (PE ones-matmuls) + K/V projection with the
     LN fold done via augmented contraction rows; writes kvflat [4097,1024]
     (row 4096 = zeros, the 3x3-window zero-padding target).
  B: per query-tile: LN of selected x/y rows (token-major, scalar-engine
     normalize), PE transposes to channel-major, Q / gate projections.
  C: windowed attention: indirect-DMA gather of K|V rows, DVE QK products
     with grouped per-head reduction, Exp softmax over 9 taps, DVE AV.
  D: proj + gating + LN2 + MLP (exact Gelu) fused per tile.
"""

import os

os.environ.setdefault("MYCRO_LOCAL_CACHE", "1")

import numpy as np

import concourse.bacc as bacc
import concourse.bass as bass
import concourse.mybir as mybir
import concourse.tile as tile
from concourse.bass import IndirectOffsetOnAxis
from concourse.bass_utils import run_bass_kernel_spmd

P = 128
C = 512          # model dim
N = 4096         # H*W tokens
NQ = 2048        # sparse queries
NT = NQ // P     # 16 query tiles
NTX = N // P     # 32 full-grid token tiles
HEADS = 16
HD = 32
KS2 = 9          # 3x3 window
C2 = 2 * C       # 1024
EPS = 1e-5
F32 = mybir.dt.float32
I32 = mybir.dt.int32
AF = mybir.ActivationFunctionType
ALU = mybir.AluOpType

N_CORES = 8


# --------------------------------------------------------------------------
# device program
# --------------------------------------------------------------------------

def build_nc(debug_out=False):
    nc = bacc.Bacc("TRN2", target_bir_lowering=False, debug=False,
                   num_devices=N_CORES)

    dt = F32
    # ---- DRAM I/O ----
    xcm = nc.dram_tensor("xcm", [P, 4, N], dt, kind="ExternalInput")
    xsel = nc.dram_tensor("xsel", [P, NT, C], dt, kind="ExternalInput")
    ysel = nc.dram_tensor("ysel", [P, NT, C], dt, kind="ExternalInput")
    nbr = nc.dram_tensor("nbr", [P, NT, KS2], I32, kind="ExternalInput")
    kv_wa = nc.dram_tensor("kv_wa", [P, 5, C2], dt, kind="ExternalInput")
    q_wa = nc.dram_tensor("q_wa", [P, 9, C], dt, kind="ExternalInput")
    g1_wa = nc.dram_tensor("g1_wa", [P, 9, C], dt, kind="ExternalInput")
    g2_w = nc.dram_tensor("g2_w", [P, 4, 2], dt, kind="ExternalInput")
    g2_b = nc.dram_tensor("g2_b", [P, 2], dt, kind="ExternalInput")
    proj_wa = nc.dram_tensor("proj_wa", [P, 5, C], dt, kind="ExternalInput")
    fc1_wa = nc.dram_tensor("fc1_wa", [P, 5, C2], dt, kind="ExternalInput")
    fc2_w = nc.dram_tensor("fc2_w", [P, 8, C], dt, kind="ExternalInput")
    rpb_b = nc.dram_tensor("rpb_b", [P, HEADS * KS2], dt, kind="ExternalInput")
    ident = nc.dram_tensor("ident", [P, P], dt, kind="ExternalInput")
    kvflat = nc.dram_tensor("kvflat", [N + 1, C2], dt)
    if debug_out:
        dq = nc.dram_tensor("dq", [P, NT, C], dt, kind="ExternalOutput")
        dg = nc.dram_tensor("dg", [P, NT, 2], dt, kind="ExternalOutput")
        dat = nc.dram_tensor("dat", [P, 4, NQ], dt, kind="ExternalOutput")
        dfu = nc.dram_tensor("dfu", [P, NT, C], dt, kind="ExternalOutput")
    outp = nc.dram_tensor("outp", [P, NT, C], dt, kind="ExternalOutput")

    with tile.TileContext(nc) as tc:
        with (
            tc.tile_pool(name="const", bufs=1) as constp,
            tc.tile_pool(name="persist", bufs=1) as persist,
        ):
            ident_sb = constp.tile([P, P], dt)
            nc.sync.dma_start(out=ident_sb[:], in_=ident[:])
            rpb_sb = constp.tile([P, HEADS * KS2], dt)
            nc.sync.dma_start(out=rpb_sb[:], in_=rpb_b[:])
            nbr_sb = constp.tile([P, NT, KS2], I32)
            nc.sync.dma_start(out=nbr_sb[:], in_=nbr[:])
            g2b_sb = constp.tile([P, 2], dt)
            nc.sync.dma_start(out=g2b_sb[:], in_=g2_b[:])
            ones_aug = constp.tile([P, P], dt)
            nc.gpsimd.memset(ones_aug[:], 0.0)
            nc.gpsimd.memset(ones_aug[0:1, :], 1.0)
            eps_sb = constp.tile([P, 1], dt)
            nc.gpsimd.memset(eps_sb[:], EPS)
            onecol = constp.tile([P, 1], dt)
            nc.gpsimd.memset(onecol[:], 1.0)

            # zero row of kvflat (padding row for out-of-bounds windows)
            zrow = constp.tile([1, C2], dt)
            nc.gpsimd.memset(zrow[:], 0.0)
            nc.sync.dma_start(out=kvflat[N:N + 1, :], in_=zrow[:])

            g_sb = persist.tile([P, NT, 2], dt)

            # --------------------------------------------------------------
            # Phase A: full-grid LN stats of x + K/V projection -> kvflat
            # --------------------------------------------------------------
            with (
                tc.tile_pool(name="pa_big", bufs=1) as pa_big,
                tc.tile_pool(name="pa_sm", bufs=2) as pa_sm,
                tc.tile_pool(name="pa_ps", bufs=2, space="PSUM") as pa_ps,
                tc.tile_pool(name="pa_ps1", bufs=2, space="PSUM") as pa_ps1,
            ):
                xcm_sb = pa_big.tile([P, 4, N], dt)
                nc.sync.dma_start(out=xcm_sb[:], in_=xcm[:])
                kvw_sb = pa_big.tile([P, 5, C2], dt)
                nc.sync.dma_start(out=kvw_sb[:], in_=kv_wa[:])

                # stats rows in [8, 512] layout: token t -> [t//512, t%512]
                s_sum = pa_big.tile([8, C], dt, tag="s_sum")
                s_sq = pa_big.tile([8, C], dt, tag="s_sq")
                for j in range(8):
                    ps_sum = pa_ps1.tile([1, C], dt, tag="ps_sum")
                    ps_sq = pa_ps1.tile([1, C], dt, tag="ps_sq")
                    for ch in range(4):
                        sl = xcm_sb[:, ch, j * C:(j + 1) * C]
                        nc.tensor.matmul(out=ps_sum[:], lhsT=onecol[:], rhs=sl,
                                         start=(ch == 0), stop=(ch == 3))
                        sqt = pa_sm.tile([P, C], dt, tag="sqt")
                        nc.scalar.square(sqt[:], sl)
                        nc.tensor.matmul(out=ps_sq[:], lhsT=onecol[:], rhs=sqt[:],
                                         start=(ch == 0), stop=(ch == 3))
                    tmp_s = pa_sm.tile([1, C], dt, tag="tmp_s")
                    nc.scalar.activation(tmp_s[:], ps_sum[:], AF.Identity,
                                         scale=1.0 / C)
                    nc.sync.dma_start(out=s_sum[j:j + 1, :], in_=tmp_s[:])
                    tmp_q = pa_sm.tile([1, C], dt, tag="tmp_q")
                    nc.scalar.activation(tmp_q[:], ps_sq[:], AF.Identity,
                                         scale=1.0 / C)
                    nc.sync.dma_start(out=s_sq[j:j + 1, :], in_=tmp_q[:])
                # var = E[x^2] - m^2 ; rinv = sqrt(var+eps) ; r = 1/rinv
                s_msq = pa_big.tile([8, C], dt, tag="s_msq")
                nc.vector.tensor_mul(s_msq[:], s_sum[:], s_sum[:])
                s_var = pa_big.tile([8, C], dt, tag="s_var")
                nc.vector.tensor_tensor(out=s_var[:], in0=s_sq[:], in1=s_msq[:],
                                        op=ALU.subtract)
                s_rinv = pa_big.tile([8, C], dt, tag="s_rinv")
                nc.scalar.activation(s_rinv[:], s_var[:], AF.Sqrt, bias=eps_sb[:8])
                s_r = pa_big.tile([8, C], dt, tag="s_r")
                nc.vector.reciprocal(s_r[:], s_rinv[:])

                # aug lhsT tile: row0 = mean row, row1 = rinv row, rest 0
                xstat = pa_big.tile([P, N], dt)
                nc.gpsimd.memset(xstat[:], 0.0)
                nc.sync.dma_start(
                    out=xstat[0:1, :].rearrange("o (s f) -> o s f", s=8),
                    in_=s_sum[:])
                nc.sync.dma_start(
                    out=xstat[1:2, :].rearrange("o (s f) -> o s f", s=8),
                    in_=s_rinv[:])

                # r in token-major [128, 32] (col = token tile)
                r_tok = pa_big.tile([P, NTX], dt)
                r_tok_v = r_tok[:].rearrange("p (s j) -> p s j", j=4)
                for j in range(4):
                    ps_t = pa_ps.tile([P, 8], dt, tag="ps_rt")
                    nc.tensor.transpose(ps_t[:], s_r[:, j * P:(j + 1) * P],
                                        ident_sb[:8, :8])
                    nc.scalar.copy(r_tok_v[:, :, j], ps_t[:])

                # K/V projection: kv[t] = r_t * (x @ W' + m*(-u) + rinv*v)
                for i in range(NTX):
                    tsl = slice(i * P, (i + 1) * P)
                    for half in range(2):
                        osl = slice(half * C, (half + 1) * C)
                        ps = pa_ps.tile([P, C], dt, tag="ps_kv")
                        for ch in range(4):
                            nc.tensor.matmul(out=ps[:],
                                             lhsT=xcm_sb[:, ch, tsl],
                                             rhs=kvw_sb[:, ch, osl],
                                             start=(ch == 0), stop=False)
                        nc.tensor.matmul(out=ps[:], lhsT=xstat[:, tsl],
                                         rhs=kvw_sb[:, 4, osl],
                                         start=False, stop=True)
                        ev = pa_sm.tile([P, C], dt, tag="kv_ev")
                        nc.scalar.activation(ev[:], ps[:], AF.Copy,
                                             scale=r_tok[:, i:i + 1])
                        nc.sync.dma_start(out=kvflat[tsl, osl], in_=ev[:])

            # --------------------------------------------------------------
            # Phase B: selected-token LN -> Q, gate (per-tile fused)
            # --------------------------------------------------------------
            with tc.tile_pool(name="qpool", bufs=1) as qpool:
                q_sb = qpool.tile([P, NT, C], dt)
                with (
                    tc.tile_pool(name="pb_w", bufs=1) as pb_w,
                    tc.tile_pool(name="pb_sm", bufs=2) as pb_sm,
                    tc.tile_pool(name="pb_ps", bufs=2, space="PSUM") as pb_ps,
                    tc.tile_pool(name="pb_ps2", bufs=2, space="PSUM") as pb_ps2,
                    tc.tile_pool(name="pb_ps3", bufs=1, space="PSUM") as pb_ps3,
                ):
                    qw_sb = pb_w.tile([P, 9, C], dt)
                    nc.sync.dma_start(out=qw_sb[:], in_=q_wa[:])
                    g1w_sb = pb_w.tile([P, 9, C], dt)
                    nc.sync.dma_start(out=g1w_sb[:], in_=g1_wa[:])
                    g2w_sb = pb_w.tile([P, 4, 2], dt)
                    nc.sync.dma_start(out=g2w_sb[:], in_=g2_w[:])

                    for i in range(NT):
                        ycat_blk = pb_sm.tile([P, 8, P], dt, tag="ycat_blk")
                        for part, seldram in ((0, xsel), (1, ysel)):
                            st = pb_sm.tile([P, C], dt, tag="sel_in")
                            nc.sync.dma_start(out=st[:], in_=seldram[:, i, :])
                            bn6 = pb_sm.tile([P, 6], dt, tag="bn6")
                            nc.vector.bn_stats(bn6[:], st[:])
                            mv = pb_sm.tile([P, 2], dt, tag="mv")
                            nc.vector.bn_aggr(mv[:], bn6[:])
                            sd = pb_sm.tile([P, 1], dt, tag="sdb")
                            nc.scalar.activation(sd[:], mv[:, 1:2], AF.Sqrt,
                                                 bias=eps_sb[:])
                            rr = pb_sm.tile([P, 1], dt, tag="rrb")
                            nc.vector.reciprocal(rr[:], sd[:])
                            mb = pb_sm.tile([P, 1], dt, tag="mb")
                            nc.vector.tensor_scalar(out=mb[:], in0=mv[:, 0:1],
                                                    scalar1=rr[:], scalar2=-1.0,
                                                    op0=ALU.mult, op1=ALU.mult)
                            xn = pb_sm.tile([P, C], dt, tag="xn")
                            nc.scalar.activation(xn[:], st[:], AF.Identity,
                                                 scale=rr[:], bias=mb[:])
                            for ch in range(4):
                                pst = pb_ps2.tile([P, P], dt, tag="ps_tr")
                                nc.tensor.transpose(
                                    pst[:], xn[:, ch * P:(ch + 1) * P],
                                    ident_sb[:])
                                nc.scalar.copy(ycat_blk[:, 4 * part + ch, :],
                                               pst[:])

                        for which, w_sb in ((0, qw_sb), (1, g1w_sb)):
                            ps1 = pb_ps.tile([P, C], dt, tag="ps_q1")
                            for ch in range(8):
                                nc.tensor.matmul(out=ps1[:],
                                                 lhsT=ycat_blk[:, ch, :],
                                                 rhs=w_sb[:, ch, :],
                                                 start=(ch == 0), stop=False)
                            nc.tensor.matmul(out=ps1[:], lhsT=ones_aug[:],
                                             rhs=w_sb[:, 8, :],
                                             start=False, stop=True)
                            if which == 0:
                                nc.scalar.copy(q_sb[:, i, :], ps1[:])
                                if debug_out:
                                    nc.sync.dma_start(out=dq[:, i, :],
                                                      in_=q_sb[:, i, :])
                            else:
                                gact = pb_sm.tile([P, C], dt, tag="gact")
                                nc.scalar.activation(gact[:], ps1[:], AF.Relu)
                                gcm = pb_sm.tile([P, 4, P], dt, tag="gcm")
                                for ch in range(4):
                                    pst = pb_ps2.tile([P, P], dt, tag="ps_tr")
                                    nc.tensor.transpose(
                                        pst[:], gact[:, ch * P:(ch + 1) * P],
                                        ident_sb[:])
                                    nc.scalar.copy(gcm[:, ch, :], pst[:])
                                psg = pb_ps3.tile([P, 2], dt, tag="ps_misc")
                                for ch in range(4):
                                    nc.tensor.matmul(out=psg[:],
                                                     lhsT=gcm[:, ch, :],
                                                     rhs=g2w_sb[:, ch, :],
                                                     start=(ch == 0),
                                                     stop=(ch == 3))
                                nc.scalar.activation(g_sb[:, i, 0:1],
                                                     psg[:, 0:1], AF.Sigmoid,
                                                     bias=g2b_sb[:, 0:1])
                                nc.scalar.activation(g_sb[:, i, 1:2],
                                                     psg[:, 1:2], AF.Sigmoid,
                                                     bias=g2b_sb[:, 1:2])
                                if debug_out:
                                    nc.sync.dma_start(out=dg[:, i, :],
                                                      in_=g_sb[:, i, :])

                # ----------------------------------------------------------
                # Phase C: windowed attention
                # ----------------------------------------------------------
                with tc.tile_pool(name="acm", bufs=1) as acmp:
                    attn_cm = acmp.tile([P, 4, NQ], dt)
                    with (
                        tc.tile_pool(name="pc_g", bufs=2) as pc_g,
                        tc.tile_pool(name="pc_sm", bufs=3) as pc_sm,
                        tc.tile_pool(name="pc_ps", bufs=2, space="PSUM") as pc_ps,
                    ):
                        for i in range(NT):
                            kvg = pc_g.tile([P, KS2, C2], dt, tag="kvg")
                            for k in range(KS2):
                                nc.gpsimd.indirect_dma_start(
                                    out=kvg[:, k, :], out_offset=None,
                                    in_=kvflat[:],
                                    in_offset=IndirectOffsetOnAxis(
                                        ap=nbr_sb[:, i, k:k + 1], axis=0))
                            logit = pc_sm.tile([P, HEADS, KS2], dt, tag="logit")
                            for k in range(KS2):
                                prod = pc_sm.tile([P, C], dt, tag="prod")
                                nc.vector.tensor_mul(prod[:], q_sb[:, i, :],
                                                     kvg[:, k, 0:C])
                                nc.vector.tensor_reduce(
                                    out=logit[:, :, k],
                                    in_=prod[:].rearrange("p (h d) -> p h d",
                                                          d=HD),
                                    axis=mybir.AxisListType.X, op=ALU.add)
                            lr = pc_sm.tile([P, HEADS * KS2], dt, tag="lr")
                            nc.vector.tensor_add(
                                lr[:], logit[:].rearrange("p h k -> p (h k)"),
                                rpb_sb[:])
                            ex = pc_sm.tile([P, HEADS, KS2], dt, tag="ex")
                            nc.scalar.activation(
                                ex[:].rearrange("p h k -> p (h k)"), lr[:],
                                AF.Exp)
                            ssum = pc_sm.tile([P, HEADS], dt, tag="ssum")
                            nc.vector.tensor_reduce(out=ssum[:], in_=ex[:],
                                                    axis=mybir.AxisListType.X,
                                                    op=ALU.add)
                            sinv = pc_sm.tile([P, HEADS], dt, tag="sinv")
                            nc.vector.reciprocal(sinv[:], ssum[:])
                            attw = pc_sm.tile([P, HEADS, KS2], dt, tag="attw")
                            nc.vector.tensor_mul(
                                attw[:], ex[:],
                                sinv[:, :, None].to_broadcast([P, HEADS, KS2]))
                            acc = pc_sm.tile([P, C], dt, tag="acc")
                            accv = acc[:].rearrange("p (h d) -> p h d", d=HD)
                            nc.vector.tensor_tensor(
                                out=accv,
                                in0=kvg[:, 0, C:].rearrange("p (h d) -> p h d",
                                                            d=HD),
                                in1=attw[:, :, 0:1].to_broadcast([P, HEADS, HD]),
                                op=ALU.mult)
                            for k in range(1, KS2):
                                avt = pc_sm.tile([P, C], dt, tag="avt")
                                nc.vector.tensor_tensor(
                                    out=avt[:].rearrange("p (h d) -> p h d",
                                                         d=HD),
                                    in0=kvg[:, k, C:].rearrange(
                                        "p (h d) -> p h d", d=HD),
                                    in1=attw[:, :, k:k + 1].to_broadcast(
                                        [P, HEADS, HD]),
                                    op=ALU.mult)
                                nc.vector.tensor_add(acc[:], acc[:], avt[:])
                            for ch in range(4):
                                pst = pc_ps.tile([P, P], dt, tag="ps_tr")
                                nc.tensor.transpose(
                                    pst[:], acc[:, ch * P:(ch + 1) * P],
                                    ident_sb[:])
                                nc.scalar.copy(
                                    attn_cm[:, ch, i * P:(i + 1) * P], pst[:])
                            if debug_out:
                                nc.sync.dma_start(
                                    out=dat[:, :, i * P:(i + 1) * P],
                                    in_=attn_cm[:, :, i * P:(i + 1) * P])

                    # ------------------------------------------------------
                    # Phase D: proj + gating + LN2 + MLP (per-tile fused)
                    # ------------------------------------------------------
                    with (
                        tc.tile_pool(name="pd_w", bufs=1) as pd_w,
                        tc.tile_pool(name="pd_sm", bufs=2) as pd_sm,
                        tc.tile_pool(name="pd_ps", bufs=2, space="PSUM") as pd_ps,
                        tc.tile_pool(name="pd_ps2", bufs=2, space="PSUM") as pd_ps2,
                    ):
                        pw_sb = pd_w.tile([P, 5, C], dt)
                        nc.sync.dma_start(out=pw_sb[:], in_=proj_wa[:])
                        f1_sb = pd_w.tile([P, 5, C2], dt)
                        nc.sync.dma_start(out=f1_sb[:], in_=fc1_wa[:])
                        f2_sb = pd_w.tile([P, 8, C], dt)
                        nc.sync.dma_start(out=f2_sb[:], in_=fc2_w[:])

                        for i in range(NT):
                            tsl = slice(i * P, (i + 1) * P)
                            ps = pd_ps.tile([P, C], dt, tag="ps_pj")
                            for ch in range(4):
                                nc.tensor.matmul(out=ps[:],
                                                 lhsT=attn_cm[:, ch, tsl],
                                                 rhs=pw_sb[:, ch, :],
                                                 start=(ch == 0), stop=False)
                            nc.tensor.matmul(out=ps[:], lhsT=ones_aug[:],
                                             rhs=pw_sb[:, 4, :],
                                             start=False, stop=True)
                            t1 = pd_sm.tile([P, C], dt, tag="pj_t1")
                            nc.scalar.activation(t1[:], ps[:], AF.Copy,
                                                 scale=g_sb[:, i, 0:1])
                            res = pd_sm.tile([P, C], dt, tag="res")
                            nc.sync.dma_start(out=res[:], in_=ysel[:, i, :])
                            t2 = pd_sm.tile([P, C], dt, tag="pj_t2")
                            nc.vector.tensor_scalar(out=t2[:], in0=res[:],
                                                    scalar1=g_sb[:, i, 1:2],
                                                    scalar2=None, op0=ALU.mult)
                            fused = pd_sm.tile([P, C], dt, tag="fused")
                            nc.vector.tensor_add(fused[:], t1[:], t2[:])
                            if debug_out:
                                nc.sync.dma_start(out=dfu[:, i, :], in_=fused[:])

                            bn6 = pd_sm.tile([P, 6], dt, tag="bn6d")
                            nc.vector.bn_stats(bn6[:], fused[:])
                            mv = pd_sm.tile([P, 2], dt, tag="mvd")
                            nc.vector.bn_aggr(mv[:], bn6[:])
                            sd = pd_sm.tile([P, 1], dt, tag="sdd")
                            nc.scalar.activation(sd[:], mv[:, 1:2], AF.Sqrt,
                                                 bias=eps_sb[:])
                            rr = pd_sm.tile([P, 1], dt, tag="rrd")
                            nc.vector.reciprocal(rr[:], sd[:])
                            mb = pd_sm.tile([P, 1], dt, tag="mbd")
                            nc.vector.tensor_scalar(out=mb[:], in0=mv[:, 0:1],
                                                    scalar1=rr[:], scalar2=-1.0,
                                                    op0=ALU.mult, op1=ALU.mult)
                            hn = pd_sm.tile([P, C], dt, tag="hn")
                            nc.scalar.activation(hn[:], fused[:], AF.Identity,
                                                 scale=rr[:], bias=mb[:])
                            hblk = pd_sm.tile([P, 4, P], dt, tag="hblk")
                            for ch in range(4):
                                pst = pd_ps2.tile([P, P], dt, tag="ps_trd")
                                nc.tensor.transpose(
                                    pst[:], hn[:, ch * P:(ch + 1) * P],
                                    ident_sb[:])
                                nc.scalar.copy(hblk[:, ch, :], pst[:])

                            a1blk = pd_sm.tile([P, 8, P], dt, tag="a1blk")
                            for half in range(2):
                                osl = slice(half * C, (half + 1) * C)
                                psf = pd_ps.tile([P, C], dt, tag="ps_f1")
                                for ch in range(4):
                                    nc.tensor.matmul(out=psf[:],
                                                     lhsT=hblk[:, ch, :],
                                                     rhs=f1_sb[:, ch, osl],
                                                     start=(ch == 0),
                                                     stop=False)
                                nc.tensor.matmul(out=psf[:], lhsT=ones_aug[:],
                                                 rhs=f1_sb[:, 4, osl],
                                                 start=False, stop=True)
                                gel = pd_sm.tile([P, C], dt, tag="gel")
                                nc.scalar.activation(gel[:], psf[:], AF.Gelu)
                                for ch in range(4):
                                    pst = pd_ps2.tile([P, P], dt, tag="ps_trd")
                                    nc.tensor.transpose(
                                        pst[:], gel[:, ch * P:(ch + 1) * P],
                                        ident_sb[:])
                                    nc.scalar.copy(a1blk[:, 4 * half + ch, :],
                                                   pst[:])

                            pso = pd_ps.tile([P, C], dt, tag="ps_f2")
                            for ch in range(8):
                                nc.tensor.matmul(out=pso[:],
                                                 lhsT=a1blk[:, ch, :],
                                                 rhs=f2_sb[:, ch, :],
                                                 start=(ch == 0),
                                                 stop=(ch == 7))
                            ot = pd_sm.tile([P, C], dt, tag="ot")
                            nc.vector.tensor_add(ot[:], pso[:], fused[:])
                            nc.sync.dma_start(out=outp[:, i, :], in_=ot[:])

    nc.compile()
    return nc


# --------------------------------------------------------------------------
# host-side prep
# --------------------------------------------------------------------------

def _cm(w):
    """[R, O] matrix -> [128, R//128, O] with row r at [r%128, r//128]."""
    r, o = w.shape
    return np.ascontiguousarray(
        w.reshape(r // P, P, o).transpose(1, 0, 2)).astype(np.float32)


def _tm(x):
    """[T, C] token-major -> [128, T//128, C] with token t=i*128+p at [p,i]."""
    t, c = x.shape
    return np.ascontiguousarray(
        x.reshape(t // P, P, c).transpose(1, 0, 2))


def _prep_core(x_img, y_img, idxb, p, gx, bx, gy, by, g2n, b2n, fc1, fc2):
    """Build the in_map for one core.

    x_img, y_img: [C, N] channel-major raw images (x = K/V source modality).
    p: sna params dict for this core's SNA block.
    """
    scale = -(HD ** -0.5)

    xsel = x_img[:, idxb].T.astype(np.float32)          # [NQ, C]
    ysel = y_img[:, idxb].T.astype(np.float32)

    r, cc = idxb // 64, idxb % 64
    nbr = np.full((NQ, KS2), N, dtype=np.int32)
    for di in range(3):
        for dj in range(3):
            rr, c2 = r + di - 1, cc + dj - 1
            ok = (rr >= 0) & (rr < 64) & (c2 >= 0) & (c2 < 64)
            nbr[:, di * 3 + dj] = np.where(ok, rr * 64 + c2, N).astype(np.int32)

    kv_wp = gx[:, None] * p["kv_w"]
    kv_aug = np.zeros((P, C2), np.float32)
    kv_aug[0] = -kv_wp.sum(0)
    kv_aug[1] = bx @ p["kv_w"] + p["kv_b"]
    kv_wa = np.concatenate([kv_wp, kv_aug], axis=0)          # [640, 1024]

    def aug2(w, b, sc):
        """[1024, O] two-LN weight -> [1152, O]: gamma-folded + bias row."""
        wx = (gx[:, None] * w[:C]) * sc
        wy = (gy[:, None] * w[C:]) * sc
        aug = np.zeros((P, w.shape[1]), np.float32)
        aug[0] = (bx @ w[:C] + by @ w[C:] + b) * sc
        return np.concatenate([wx, wy, aug], axis=0)

    q_wa = aug2(p["q_w"], p["q_b"], scale)
    g1_wa = aug2(p["g1_w"], p["g1_b"], 1.0)

    proj_aug = np.zeros((P, C), np.float32)
    proj_aug[0] = p["proj_b"]
    proj_wa = np.concatenate([p["proj_w"], proj_aug], axis=0)    # [640, 512]

    fc1_p = g2n[:, None] * fc1
    fc1_aug = np.zeros((P, C2), np.float32)
    fc1_aug[0] = b2n @ fc1
    fc1_wa = np.concatenate([fc1_p, fc1_aug], axis=0)            # [640, 1024]

    rpb_b = np.broadcast_to(
        p["rpb"].reshape(1, HEADS * KS2), (P, HEADS * KS2)).copy()

    return {
        "xcm": _cm(x_img),
        "xsel": _tm(xsel), "ysel": _tm(ysel),
        "nbr": _tm(nbr).astype(np.int32),
        "kv_wa": _cm(kv_wa), "q_wa": _cm(q_wa), "g1_wa": _cm(g1_wa),
        "g2_w": _cm(p["g2_w"]),
        "g2_b": np.broadcast_to(p["g2_b"].reshape(1, 2), (P, 2))
        .astype(np.float32).copy(),
        "proj_wa": _cm(proj_wa), "fc1_wa": _cm(fc1_wa), "fc2_w": _cm(fc2),
        "rpb_b": rpb_b.astype(np.float32),
        "ident": np.eye(P, dtype=np.float32),
    }


_NC = None


def _get_nc():
    global _NC
    if _NC is None:
        _NC = build_nc()
    return _NC


def kernel(input_rgb, input_thermal, idx, params, _trace=False):
    input_rgb = np.asarray(input_rgb, np.float32)
    input_thermal = np.asarray(input_thermal, np.float32)
    idx = np.asarray(idx, np.int32)
    params = {k: ({kk: np.asarray(vv, np.float32) for kk, vv in v.items()}
                  if isinstance(v, dict) else np.asarray(v, np.float32))
              for k, v in params.items()}

    B = input_rgb.shape[0]
    rgb_cm = input_rgb.reshape(B, C, N)
    th_cm = input_thermal.reshape(B, C, N)

    in_maps = []
    for core in range(N_CORES):
        b, mod = core // 2, core % 2
        if mod == 0:   # fuse_rgb: x = thermal (K/V src), y = rgb
            m = _prep_core(th_cm[b], rgb_cm[b], idx[b], params["sna_rgb"],
                           params["th_norm1_g"], params["th_norm1_b"],
                           params["rgb_norm1_g"], params["rgb_norm1_b"],
                           params["rgb_norm2_g"], params["rgb_norm2_b"],
                           params["rgb_mlp_fc1_w"], params["rgb_mlp_fc2_w"])
        else:          # fuse_th: x = rgb, y = thermal
            m = _prep_core(rgb_cm[b], th_cm[b], idx[b], params["sna_th"],
                           params["rgb_norm1_g"], params["rgb_norm1_b"],
                           params["th_norm1_g"], params["th_norm1_b"],
                           params["th_norm2_g"], params["th_norm2_b"],
                           params["th_mlp_fc1_w"], params["th_mlp_fc2_w"])
        in_maps.append(m)

    nc = _get_nc()
    res = run_bass_kernel_spmd(nc, in_maps, core_ids=list(range(N_CORES)),
                               trace=_trace)

    rgb_full = np.zeros((B, C, N), np.float32)
    th_full = np.zeros((B, C, N), np.float32)
    for core in range(N_CORES):
        b, mod = core // 2, core % 2
        o = res.results[core]["outp"]            # [128, 16, 512]
        out_sel = o.transpose(1, 0, 2).reshape(NQ, C)
        tgt = rgb_full if mod == 0 else th_full
        tgt[b][:, idx[b]] = out_sel.T
    rgb_out = rgb_full.reshape(B, C, 64, 64)
    th_out = th_full.reshape(B, C, 64, 64)
    if _trace:
        return (rgb_out, th_out), res
    return (rgb_out, th_out)
